# revision 7
# baseline (speedup 1.0000x reference)
"""Trainium2 Bass kernel for nn_DotPredictor — dma_gather version.

score[e] = <h[src[e]], h[dst[e]]>   h: [100000, 128] f32, src/dst: [640000] int

Strategy (8 NeuronCores, SPMD):
  - h converted to bf16 on host (free; rel-err budget 2e-2 >> bf16 noise).
  - Edges bucketed host-side into 16 classes by (src//25000, dst//25000) so
    both gathers of a class read one 25000-row chunk — local indices fit the
    int16 that InstDMAGatherAnt requires. Each class is split evenly over the
    8 cores; per-(core,class) groups padded to a shared capacity (idx 0).
  - Per class: dma_gather batches of <=1024 rows (SWDGE ring carveout limit,
    probed: 2048 wedges the device) on rotating SWDGE queues. One instruction
    gathers 1024 rows vs 128 for indirect_dma_start -> 8x less desc-gen
    fixed overhead on the Pool engine.
  - Gathered batch i lands [p=i%128, col=i//128]; multiply (bf16, DVE) +
    free-axis reduce -> score[128, SLOTS/128] f32; host unpermutes.
"""

import numpy as np
import ml_dtypes

N_NODES = 100000
N_EDGES = 640000
D = 128
N_CORES = 8
NCHUNK = 4
CH = N_NODES // NCHUNK  # 25000 rows per chunk, local idx < 32768
BATCH = 1024  # rows per dma_gather instruction (ring carveout limit)
NQ = 4  # SWDGE queues
WORK_BUFS = 6
DTYPE = "bf16"
SORT_GROUPS = False

_cache = {}


def _build(capacities):
    key = (tuple(capacities), DTYPE, NQ, WORK_BUFS)
    if key in _cache:
        return _cache[key]

    from concourse import bacc, mybir
    import concourse.tile as tile

    slots = int(sum(capacities))
    n_idx_cols = slots // 16
    n_score_cols = slots // 128

    dt = mybir.dt.bfloat16 if DTYPE == "bf16" else mybir.dt.float32
    nc = bacc.Bacc("TRN2", num_swdge_queues=NQ)
    h_ext = nc.dram_tensor("h", [N_NODES, D], dt, kind="ExternalInput")
    src_ext = nc.dram_tensor(
        "src_idx", [128, n_idx_cols], mybir.dt.int16, kind="ExternalInput"
    )
    dst_ext = nc.dram_tensor(
        "dst_idx", [128, n_idx_cols], mybir.dt.int16, kind="ExternalInput"
    )
    score_ext = nc.dram_tensor(
        "score", [128, n_score_cols], mybir.dt.float32, kind="ExternalOutput"
    )

    with tile.TileContext(nc) as tc:
        with (
            tc.tile_pool(name="idx", bufs=1) as idx_pool,
            tc.tile_pool(name="work", bufs=WORK_BUFS) as work_pool,
            tc.tile_pool(name="acc", bufs=1) as acc_pool,
        ):
            src_t = idx_pool.tile([128, n_idx_cols], mybir.dt.int16, tag="src")
            dst_t = idx_pool.tile([128, n_idx_cols], mybir.dt.int16, tag="dst")
            nc.sync.dma_start(out=src_t[:], in_=src_ext[:])
            nc.sync.dma_start(out=dst_t[:], in_=dst_ext[:])
            score_t = acc_pool.tile([128, n_score_cols], mybir.dt.float32, tag="sc")

            qn = 0
            off = 0
            for g, cap in enumerate(capacities):
                cs, cd = divmod(g, NCHUNK)
                h_src = h_ext[cs * CH : (cs + 1) * CH, :]
                h_dst = h_ext[cd * CH : (cd + 1) * CH, :]
                done = 0
                while done < cap:
                    n = min(BATCH, cap - done)
                    s0 = off + done
                    b = n // 128
                    hu = work_pool.tile([128, n], dt, tag=f"hu{n}")
                    hv = work_pool.tile([128, n], dt, tag=f"hv{n}")
                    nc.gpsimd.dma_gather(
                        hu[:].rearrange("p (b d) -> p b d", d=D),
                        h_src,
                        src_t[:, s0 // 16 : (s0 + n) // 16],
                        n,
                        n,
                        D,
                        queue_num=qn % NQ,
                    )
                    qn += 1
                    nc.gpsimd.dma_gather(
                        hv[:].rearrange("p (b d) -> p b d", d=D),
                        h_dst,
                        dst_t[:, s0 // 16 : (s0 + n) // 16],
                        n,
                        n,
                        D,
                        queue_num=qn % NQ,
                    )
                    qn += 1
                    nc.vector.tensor_tensor(
                        out=hu[:], in0=hu[:], in1=hv[:], op=mybir.AluOpType.mult
                    )
                    nc.vector.tensor_reduce(
                        out=score_t[:, s0 // 128 : s0 // 128 + b],
                        in_=hu[:].rearrange("p (b d) -> p b d", d=D),
                        axis=mybir.AxisListType.X,
                        op=mybir.AluOpType.add,
                    )
                    done += n
                off += cap
            nc.sync.dma_start(out=score_ext[:], in_=score_t[:])

    nc.compile()
    _cache[key] = nc
    return nc


def _pack_inputs(h, src, dst):
    """Bucket edges into 16 chunk-classes, split over cores, build int16 idx
    tiles + unpermute maps. Returns (capacities, in_maps, gather_info)."""
    np_dt = ml_dtypes.bfloat16 if DTYPE == "bf16" else np.float32
    hb = np.ascontiguousarray(np.asarray(h), dtype=np_dt)
    src = np.asarray(src).astype(np.int64)
    dst = np.asarray(dst).astype(np.int64)

    cls = (src // CH) * NCHUNK + (dst // CH)
    order = np.argsort(cls, kind="stable")
    cls_sorted = cls[order]
    bounds = np.searchsorted(cls_sorted, np.arange(NCHUNK * NCHUNK + 1))

    # per (class, core) edge-id groups
    groups = [[None] * N_CORES for _ in range(NCHUNK * NCHUNK)]
    for g in range(NCHUNK * NCHUNK):
        ids_g = order[bounds[g] : bounds[g + 1]]
        parts = np.array_split(ids_g, N_CORES)
        for c in range(N_CORES):
            ids = parts[c]
            if SORT_GROUPS and len(ids):
                ids = ids[np.argsort(src[ids], kind="stable")]
            groups[g][c] = ids

    capacities = []
    for g in range(NCHUNK * NCHUNK):
        need = max(len(groups[g][c]) for c in range(N_CORES))
        capacities.append(max(128, -(-need // 128) * 128))
    slots = sum(capacities)
    n_idx_cols = slots // 16
    n_score_cols = slots // 128

    in_maps = []
    unperm = []  # per core: (edge_ids, linear positions into score flat)
    for c in range(N_CORES):
        src16 = np.zeros((128, n_idx_cols), np.int16)
        dst16 = np.zeros((128, n_idx_cols), np.int16)
        eids_all = []
        lin_all = []
        off = 0
        for g, cap in enumerate(capacities):
            cs, cd = divmod(g, NCHUNK)
            ids = groups[g][c]
            n = len(ids)
            ls = np.zeros(cap, np.int16)
            ld = np.zeros(cap, np.int16)
            ls[:n] = (src[ids] - cs * CH).astype(np.int16)
            ld[:n] = (dst[ids] - cd * CH).astype(np.int16)
            k = np.arange(cap)
            j = k // BATCH
            k2 = k % BATCH
            icol = (off + j * BATCH) // 16 + k2 // 16
            irow = k2 % 16
            src16[irow, icol] = ls
            dst16[irow, icol] = ld
            if n:
                col = (off + j[:n] * BATCH) // 128 + k2[:n] // 128
                p = k2[:n] % 128
                eids_all.append(ids)
                lin_all.append(p * n_score_cols + col)
            off += cap
        src16[16:] = np.tile(src16[:16], (7, 1))
        dst16[16:] = np.tile(dst16[:16], (7, 1))
        in_maps.append({"h": hb, "src_idx": src16, "dst_idx": dst16})
        unperm.append((np.concatenate(eids_all), np.concatenate(lin_all)))
    return capacities, in_maps, unperm


def _prepare(h, src, dst):
    capacities, in_maps, unperm = _pack_inputs(h, src, dst)
    nc = _build(capacities)
    return nc, in_maps, unperm


def kernel(h, src, dst):
    nc, in_maps, unperm = _prepare(h, src, dst)
    from concourse.bass_utils import run_bass_kernel_spmd

    res = run_bass_kernel_spmd(nc, in_maps, list(range(N_CORES)))
    out = np.zeros(N_EDGES, np.float32)
    for c in range(N_CORES):
        score = np.asarray(res.results[c]["score"]).reshape(-1)
        eids, lin = unperm[c]
        out[eids] = score[lin]
    return out


# revision 8
# speedup vs baseline: 10.9116x; 10.9116x over previous
"""Trainium2 Bass kernel for nn_DotPredictor — dma_gather version.

score[e] = <h[src[e]], h[dst[e]]>   h: [100000, 128] f32, src/dst: [640000] int

Strategy (8 NeuronCores, SPMD):
  - h converted to bf16 on host (free; rel-err budget 2e-2 >> bf16 noise).
  - Edges bucketed host-side into 16 classes by (src//25000, dst//25000) so
    both gathers of a class read one 25000-row chunk — local indices fit the
    int16 that InstDMAGatherAnt requires. Each class is split evenly over the
    8 cores; per-(core,class) groups padded to a shared capacity (idx 0).
  - Per class: dma_gather batches of <=1024 rows (SWDGE ring carveout limit,
    probed: 2048 wedges the device) on rotating SWDGE queues. One instruction
    gathers 1024 rows vs 128 for indirect_dma_start -> 8x less desc-gen
    fixed overhead on the Pool engine.
  - Gathered batch i lands [p=i%128, col=i//128]; multiply (bf16, DVE) +
    free-axis reduce -> score[128, SLOTS/128] f32; host unpermutes.

Performance model (measured): exec ~= total_rows_gathered x 6.5ns / 4 queues
~= 264us/core. Per-SWDGE-queue descriptor throughput is the binding resource
(payload-size independent: f32 == bf16; queue scaling 1->2->4 verified).
Round-robin queue rotation is exactly balanced (40576 rows/queue). Fewer
descriptors is impossible for random 2-rows-per-edge gathers (PE one-hot
selection costs >=1 matmul per 128-edge x 128-row tile pair; random indices
make those pairs singletons), and 4 queues is the ucode maximum.
"""

import numpy as np
import ml_dtypes

N_NODES = 100000
N_EDGES = 640000
D = 128
N_CORES = 8
NCHUNK = 4
CH = N_NODES // NCHUNK  # 25000 rows per chunk, local idx < 32768
BATCH = 1024  # rows per dma_gather instruction (ring carveout limit)
NQ = 4  # SWDGE queues
WORK_BUFS = 6
DTYPE = "bf16"
SORT_GROUPS = False

_cache = {}


def _build(capacities):
    key = (tuple(capacities), DTYPE, NQ, WORK_BUFS)
    if key in _cache:
        return _cache[key]

    from concourse import bacc, mybir
    import concourse.tile as tile

    slots = int(sum(capacities))
    n_idx_cols = slots // 16
    n_score_cols = slots // 128

    dt = mybir.dt.bfloat16 if DTYPE == "bf16" else mybir.dt.float32
    nc = bacc.Bacc("TRN2", num_swdge_queues=NQ)
    h_ext = nc.dram_tensor("h", [N_NODES, D], dt, kind="ExternalInput")
    src_ext = nc.dram_tensor(
        "src_idx", [128, n_idx_cols], mybir.dt.int16, kind="ExternalInput"
    )
    dst_ext = nc.dram_tensor(
        "dst_idx", [128, n_idx_cols], mybir.dt.int16, kind="ExternalInput"
    )
    score_ext = nc.dram_tensor(
        "score", [128, n_score_cols], mybir.dt.float32, kind="ExternalOutput"
    )

    with tile.TileContext(nc) as tc:
        with (
            tc.tile_pool(name="idx", bufs=1) as idx_pool,
            tc.tile_pool(name="work", bufs=WORK_BUFS) as work_pool,
            tc.tile_pool(name="acc", bufs=1) as acc_pool,
        ):
            src_t = idx_pool.tile([128, n_idx_cols], mybir.dt.int16, tag="src")
            dst_t = idx_pool.tile([128, n_idx_cols], mybir.dt.int16, tag="dst")
            nc.sync.dma_start(out=src_t[:], in_=src_ext[:])
            nc.sync.dma_start(out=dst_t[:], in_=dst_ext[:])
            score_t = acc_pool.tile([128, n_score_cols], mybir.dt.float32, tag="sc")

            qn = 0
            off = 0
            for g, cap in enumerate(capacities):
                cs, cd = divmod(g, NCHUNK)
                h_src = h_ext[cs * CH : (cs + 1) * CH, :]
                h_dst = h_ext[cd * CH : (cd + 1) * CH, :]
                done = 0
                while done < cap:
                    n = min(BATCH, cap - done)
                    s0 = off + done
                    b = n // 128
                    hu = work_pool.tile([128, n], dt, tag=f"hu{n}")
                    hv = work_pool.tile([128, n], dt, tag=f"hv{n}")
                    nc.gpsimd.dma_gather(
                        hu[:].rearrange("p (b d) -> p b d", d=D),
                        h_src,
                        src_t[:, s0 // 16 : (s0 + n) // 16],
                        n,
                        n,
                        D,
                        queue_num=qn % NQ,
                    )
                    qn += 1
                    nc.gpsimd.dma_gather(
                        hv[:].rearrange("p (b d) -> p b d", d=D),
                        h_dst,
                        dst_t[:, s0 // 16 : (s0 + n) // 16],
                        n,
                        n,
                        D,
                        queue_num=qn % NQ,
                    )
                    qn += 1
                    nc.vector.tensor_tensor(
                        out=hu[:], in0=hu[:], in1=hv[:], op=mybir.AluOpType.mult
                    )
                    nc.vector.tensor_reduce(
                        out=score_t[:, s0 // 128 : s0 // 128 + b],
                        in_=hu[:].rearrange("p (b d) -> p b d", d=D),
                        axis=mybir.AxisListType.X,
                        op=mybir.AluOpType.add,
                    )
                    done += n
                off += cap
            nc.sync.dma_start(out=score_ext[:], in_=score_t[:])

    nc.compile()
    _cache[key] = nc
    return nc


def _pack_inputs(h, src, dst):
    """Bucket edges into 16 chunk-classes, split over cores, build int16 idx
    tiles + unpermute maps. Returns (capacities, in_maps, gather_info)."""
    np_dt = ml_dtypes.bfloat16 if DTYPE == "bf16" else np.float32
    hb = np.ascontiguousarray(np.asarray(h), dtype=np_dt)
    src = np.asarray(src).astype(np.int64)
    dst = np.asarray(dst).astype(np.int64)

    cls = (src // CH) * NCHUNK + (dst // CH)
    order = np.argsort(cls, kind="stable")
    cls_sorted = cls[order]
    bounds = np.searchsorted(cls_sorted, np.arange(NCHUNK * NCHUNK + 1))

    # per (class, core) edge-id groups
    groups = [[None] * N_CORES for _ in range(NCHUNK * NCHUNK)]
    for g in range(NCHUNK * NCHUNK):
        ids_g = order[bounds[g] : bounds[g + 1]]
        parts = np.array_split(ids_g, N_CORES)
        for c in range(N_CORES):
            ids = parts[c]
            if SORT_GROUPS and len(ids):
                ids = ids[np.argsort(src[ids], kind="stable")]
            groups[g][c] = ids

    capacities = []
    for g in range(NCHUNK * NCHUNK):
        need = max(len(groups[g][c]) for c in range(N_CORES))
        capacities.append(max(128, -(-need // 128) * 128))
    slots = sum(capacities)
    n_idx_cols = slots // 16
    n_score_cols = slots // 128

    in_maps = []
    unperm = []  # per core: (edge_ids, linear positions into score flat)
    for c in range(N_CORES):
        src16 = np.zeros((128, n_idx_cols), np.int16)
        dst16 = np.zeros((128, n_idx_cols), np.int16)
        eids_all = []
        lin_all = []
        off = 0
        for g, cap in enumerate(capacities):
            cs, cd = divmod(g, NCHUNK)
            ids = groups[g][c]
            n = len(ids)
            ls = np.zeros(cap, np.int16)
            ld = np.zeros(cap, np.int16)
            ls[:n] = (src[ids] - cs * CH).astype(np.int16)
            ld[:n] = (dst[ids] - cd * CH).astype(np.int16)
            k = np.arange(cap)
            j = k // BATCH
            k2 = k % BATCH
            icol = (off + j * BATCH) // 16 + k2 // 16
            irow = k2 % 16
            src16[irow, icol] = ls
            dst16[irow, icol] = ld
            if n:
                col = (off + j[:n] * BATCH) // 128 + k2[:n] // 128
                p = k2[:n] % 128
                eids_all.append(ids)
                lin_all.append(p * n_score_cols + col)
            off += cap
        src16[16:] = np.tile(src16[:16], (7, 1))
        dst16[16:] = np.tile(dst16[:16], (7, 1))
        in_maps.append({"h": hb, "src_idx": src16, "dst_idx": dst16})
        unperm.append((np.concatenate(eids_all), np.concatenate(lin_all)))
    return capacities, in_maps, unperm


def _prepare(h, src, dst):
    capacities, in_maps, unperm = _pack_inputs(h, src, dst)
    nc = _build(capacities)
    return nc, in_maps, unperm


def kernel(h, src, dst):
    nc, in_maps, unperm = _prepare(h, src, dst)
    from concourse.bass_utils import run_bass_kernel_spmd

    res = run_bass_kernel_spmd(nc, in_maps, list(range(N_CORES)))
    out = np.zeros(N_EDGES, np.float32)
    for c in range(N_CORES):
        score = np.asarray(res.results[c]["score"]).reshape(-1)
        eids, lin = unperm[c]
        out[eids] = score[lin]
    return out
